# revision 25
# baseline (speedup 1.0000x reference)
"""Causal single-head attention on 8 trn2 cores, batch-data-parallel.

Computes, for each batch item b:
    Q = x[b] @ Wq + bq; K = x[b] @ Wk + bk; V = x[b] @ Wv + bv
    out[b] = softmax(causal_mask(Q K^T / sqrt(H))) @ V

Shapes: x [256, 256, 384], W* [384, 64], b* [64], out [256, 256, 64] fp32.
Sharding: batch axis split across 8 cores (32 items each), weights replicated.
The host feeds x transposed ([C, B*T] layout, bf16) so the kernel needs no
on-device transposition of x (contraction dim C must sit on SBUF partitions).

All matmul operands are bf16 (halves HBM traffic for x, enables fast weight
loads); PSUM accumulation stays fp32. Max elementwise error ~4e-3 vs the fp32
reference, well under the 2e-2 gate.

Batch items are processed in pairs so the projection matmuls stream N=512.
Two-stage software pipeline keeps the PE >90% busy: iteration p emits
  proj(p):       Q/K/V projection matmuls for pair p
  attn_back(p-2): out-matmuls + 1/den normalization + output DMA
  attn_front(p-1): scores, exp+causal-zeroing, V transposes
so the scalar-exp -> gpsimd-zero chain for a pair has a full iteration of
slack before its out-matmuls, and LDWEIGHTS pull-ahead works on the small
matmuls. x^T pair tiles are DMA-prefetched XT_AHEAD pairs ahead; the xt(0)
DMA is issued before everything else (each dma_start costs ~650ns of sync
queue time) and dummy warmup/bridge matmuls keep the PE HAM clock-gate at
2.4 GHz through the initial DMA ramp.

Per pair:
  qv psum [128,512] = rows 0:64 Q^T, rows 64:128 V^T (lhsT=[Wq|Wv], rhs=x^T)
  k  psum [64,512]  = K^T  (lhsT zero-padded to M=128 so FWL stays enabled)
  qv2/k2 sbuf (bf16) = psum + per-partition bias (one DVE op / one ACT op)
Per batch item:
  sT psum [128,3,128] = scores^T blocks [s0,t0],[s0,t1],[s1,t1] (the causally
       dead [s1,t0] block is never computed)
  W = exp(SCALE*sT) straight from PSUM on the scalar engine (bf16), then the
       below-diagonal halves of both diagonal blocks are zeroed in place by
       ONE strided-AP gpsimd affine_select (idle engine, off the DVE)
  V' = [V | 1 | 1] natural layout via PE transposes of V^T; the ones columns
       live in a persistent 6-tile ring (memset once) and make the out matmul
       also produce the softmax denominator (wei @ 1)
  out' = wei^T.T @ V' -> [t, 64 | den | den];  out = out'[:,0:64] * (1/den)
"""

import numpy as np
import ml_dtypes

import concourse.bacc as bacc
import concourse.mybir as mybir
import concourse.tile as tile
from concourse import bass_utils
from concourse.masks import make_identity

N_CORES = 8
B_FULL, T, C, H = 256, 256, 384, 64
B_SHARD = B_FULL // N_CORES  # 32
N_PAIRS = B_SHARD // 2  # 16
F32 = mybir.dt.float32
BF16 = mybir.dt.bfloat16
SCALE = float(H) ** -0.5  # folded into exp: wei = exp(SCALE * scores)

EXP = mybir.ActivationFunctionType.Exp
IS_GE = mybir.AluOpType.is_ge

XT_AHEAD = 4  # pairs of x^T prefetched ahead of the projection stream
WARMUP_MM = 0  # dummy HAM-ramp matmuls before pair 0 (0 = ramp on real work)


def _build():
    nc = bacc.Bacc("TRN2", target_bir_lowering=False, debug=False, num_devices=N_CORES)

    xT_d = nc.dram_tensor("xT", [C, B_SHARD * T], BF16, kind="ExternalInput").ap()
    wq_d = nc.dram_tensor("wq", [C, H], BF16, kind="ExternalInput").ap()
    wk_d = nc.dram_tensor("wk", [C, H], BF16, kind="ExternalInput").ap()
    wv_d = nc.dram_tensor("wv", [C, H], BF16, kind="ExternalInput").ap()
    bq_d = nc.dram_tensor("bq", [H, 1], F32, kind="ExternalInput").ap()
    bk_d = nc.dram_tensor("bk", [H, 1], F32, kind="ExternalInput").ap()
    bv_d = nc.dram_tensor("bv", [H, 1], F32, kind="ExternalInput").ap()
    # partition-major output layout: row p holds every pair's [item, n, h]
    # slice for that partition, so each pair's writeback is one contiguous
    # 1KB segment per partition (4x bigger DMA packets than t-major rows).
    # The host de-swizzles back to [B_SHARD*T, H] for free.
    out_d = nc.dram_tensor(
        "out", [128, N_PAIRS * 2 * 2 * H], F32, kind="ExternalOutput"
    ).ap()

    # x^T per batch pair: [p=c%128, k=c//128, t2=512]
    xT_r = xT_d.rearrange("(k p) (b t) -> b p k t", p=128, b=N_PAIRS)
    # weights: [p=c%128, k=c//128, h]
    wq_r = wq_d.rearrange("(k p) h -> p k h", p=128)
    wk_r = wk_d.rearrange("(k p) h -> p k h", p=128)
    wv_r = wv_d.rearrange("(k p) h -> p k h", p=128)
    # out per pair: element [b, p, i, n, h] is batch item b*2+i, t = n*128+p
    out_r = out_d.rearrange("p (b i n h) -> b p i n h", b=N_PAIRS, i=2, n=2)

    with tile.TileContext(nc) as tc:
        with (
            tc.tile_pool(name="singles", bufs=1) as singles,
            tc.tile_pool(name="sb", bufs=3) as sb,
            tc.tile_pool(name="sbw", bufs=6) as sbw,
            tc.tile_pool(name="sbx", bufs=XT_AHEAD + 1) as sbx,
            tc.tile_pool(name="ps_qv", bufs=2, space="PSUM") as ps_qv,
            tc.tile_pool(name="ps_k", bufs=1, space="PSUM") as ps_k,
            tc.tile_pool(name="ps_s", bufs=2, space="PSUM") as ps_s,
            tc.tile_pool(name="ps_v", bufs=1, space="PSUM") as ps_v,
            tc.tile_pool(name="ps_o", bufs=2, space="PSUM") as ps_o,
        ):
            # ---- DMA issue order: xt(0) first (the critical path), then the
            # weights, then the rest of the prefetch, biases last. Each
            # dma_start costs ~650ns on the sync queue, so order matters.
            xts = {}

            def load_xt(bp):
                t = sbx.tile([128, 3, 512], BF16, tag="xt")
                nc.sync.dma_start(t[:], xT_r[bp])
                xts[bp] = t

            wqv = singles.tile([128, 3, 128], BF16)
            wkk = singles.tile([128, 3, 128], BF16)
            nc.vector.memset(wkk[:], 0.0)
            load_xt(0)
            nc.sync.dma_start(wqv[:, :, 0:64], wq_r)
            nc.sync.dma_start(wqv[:, :, 64:128], wv_r)
            nc.sync.dma_start(wkk[:, :, 0:64], wk_r)
            load_xt(1)
            load_xt(2)
            # per-partition bias vectors: [bq ; bv] and [bk]
            bqv_t = singles.tile([128, 1], F32)
            bk_t = singles.tile([64, 1], F32)
            nc.sync.dma_start(bqv_t[0:64, :], bq_d[:])
            nc.sync.dma_start(bqv_t[64:128, :], bv_d[:])
            nc.sync.dma_start(bk_t[:], bk_d[:])
            for bp in range(3, XT_AHEAD):
                load_xt(bp)

            # ---- one-time setup (no DMA involved) ----
            identf = singles.tile([128, 128], F32)
            make_identity(nc, identf[:])
            ident = singles.tile([128, 128], BF16)
            nc.vector.tensor_copy(ident[:], identf[:])

            # v_sb ring: the ones columns (64:66) are written once here and
            # survive reuse, since the per-item copy only touches cols 0:64.
            v_ring = [
                singles.tile([128, 2, 66], BF16, name=f"v_sb{j}", tag=f"v_sb{j}")
                for j in range(6)
            ]
            for v in v_ring:
                nc.vector.memset(v[:, :, 64:66], 1.0)
            v_ring_i = [0]

            # HAM warmup: the PE clock-gate only opens (1.2 -> 2.4 GHz) after a
            # ~3.4us window of sustained matmul activity. Burn dummy matmuls
            # during the initial DMA wait so the real stream runs warm.
            wu = singles.tile([128, 256], BF16)
            nc.vector.memset(wu[:], 0.0)
            wu_ps = ps_s.tile([128, 256], F32, tag="s_ps")
            for _ in range(WARMUP_MM):
                nc.tensor.matmul(wu_ps[:], wu[:, 0:128], wu[:], start=True, stop=True)
            # (pairs 0-1 themselves ramp the HAM clock; running them cold costs
            # ~2.6us but starts ~3us earlier and delays the ~44us full-power
            # window so it covers the stream tail. WARMUP_MM=0 disables the
            # dummy ramp; the bridge matmuls below still prevent re-throttles.)

            def proj(bp):
                xt = xts.pop(bp)
                qv_ps = ps_qv.tile([128, 512], F32, tag="qv_ps")
                k_ps = ps_k.tile([128, 512], F32, tag="k_ps")
                for c in range(3):
                    nc.tensor.matmul(
                        qv_ps[:], wqv[:, c, :], xt[:, c, :], start=(c == 0), stop=(c == 2)
                    )
                for c in range(3):
                    nc.tensor.matmul(
                        k_ps[:], wkk[:, c, :], xt[:, c, :], start=(c == 0), stop=(c == 2)
                    )
                qv2 = sb.tile([128, 512], BF16, tag="qv2")
                k2 = sb.tile([64, 512], BF16, tag="k2")
                # psum->sbuf bias-add copies, split across DVE and ACT to
                # balance engine load (both are near the critical path)
                nc.vector.tensor_scalar_add(qv2[:], qv_ps[:], bqv_t[:])
                nc.scalar.add(k2[:], k_ps[0:64, :], bk_t[:])
                return qv2, k2

            def attn_front(bp, qv2, k2):
                """Scores, exp+mask, and V transposes for both items of a pair.
                The out-matmuls run one iteration later (attn_back), so the
                exp -> gpsimd-zero chain has a full iteration of slack and the
                PE's LDWEIGHTS pull-ahead works on the out matmuls."""
                items = []
                for bi in range(2):
                    toff = bi * 256
                    qT = qv2[0:64, toff : toff + 256]
                    kT = k2[0:64, toff : toff + 256]

                    # scores^T [s, t] blocks: 0=[s0,t0] 1=[s0,t1] 2=[s1,t1]
                    s_ps = ps_s.tile([128, 3, 128], F32, tag="s_ps")
                    nc.tensor.matmul(
                        s_ps[:, 0:2, :], kT[:, 0:128], qT, start=True, stop=True
                    )
                    nc.tensor.matmul(
                        s_ps[:, 2, :], kT[:, 128:256], qT[:, 128:256], start=True, stop=True
                    )

                    # V natural [s, h] into the ring tile (ones cols persist)
                    v_ps = ps_v.tile([128, 2, 64], BF16, tag="v_ps")
                    for sh in range(2):
                        nc.tensor.transpose(
                            v_ps[:, sh, :],
                            qv2[64:128, toff + sh * 128 : toff + (sh + 1) * 128],
                            ident[64:128, 64:128],
                        )

                    # wei^T = exp(SCALE*scores^T) straight from PSUM (bf16 out),
                    # then one strided-AP affine_select zeroes the below-diag
                    # halves of BOTH diagonal blocks.
                    W = sbw.tile([128, 3, 128], BF16, tag="W")
                    nc.scalar.activation(W[:], s_ps[:], EXP, scale=SCALE)
                    nc.gpsimd.affine_select(
                        out=W[:, 0:3:2, :],
                        in_=W[:, 0:3:2, :],
                        compare_op=IS_GE,
                        fill=0.0,
                        base=0,
                        pattern=[[0, 2], [1, 128]],  # keep where (-s + t) >= 0
                        channel_multiplier=-1,
                    )

                    v_sb = v_ring[v_ring_i[0] % 6]
                    v_ring_i[0] += 1
                    nc.scalar.copy(v_sb[:, :, 0:64], v_ps[:])
                    items.append((W, v_sb))
                return items

            def attn_back(bp, items, split_dma=False):
                osb = sb.tile([128, 2, 2, 64], F32, tag="osb")
                for bi in range(2):
                    W, v_sb = items[bi]
                    # out' = wei^T.T @ [V|1|1] -> [t, 64 | den | den]
                    o_ps = ps_o.tile([128, 2, 66], F32, tag="o_ps")
                    nc.tensor.matmul(o_ps[:, 0, :], W[:, 0, :], v_sb[:, 0, :], start=True, stop=True)
                    nc.tensor.matmul(o_ps[:, 1, :], W[:, 1, :], v_sb[:, 0, :], start=True, stop=False)
                    nc.tensor.matmul(o_ps[:, 1, :], W[:, 2, :], v_sb[:, 1, :], start=False, stop=True)

                    rden = sb.tile([128, 2], F32, tag="rden")
                    nc.vector.reciprocal(rden[:], o_ps[:, :, 64])
                    nc.vector.tensor_scalar_mul(osb[:, bi, 0, :], o_ps[:, 0, 0:64], rden[:, 0:1])
                    nc.vector.tensor_scalar_mul(osb[:, bi, 1, :], o_ps[:, 1, 0:64], rden[:, 1:2])
                    if split_dma:
                        # final pair: overlap item 0's writeback with item 1
                        nc.sync.dma_start(out_r[bp, :, bi], osb[:, bi])
                if not split_dma:
                    nc.sync.dma_start(out_r[bp], osb[:])

            # ---- software-pipelined main loop (2-stage attention):
            # iteration p emits proj(p), out-matmuls(p-2), scores/exp(p-1)
            prev_proj = None
            prev_front = None
            for bp in range(N_PAIRS):
                if bp + XT_AHEAD < N_PAIRS:
                    load_xt(bp + XT_AHEAD)
                cur = proj(bp)
                if bp <= 1:
                    # bridge the early DMA ramp: pairs 0-2 are input-bound, and
                    # a PE idle window here re-throttles the HAM clock to 1.2
                    # GHz for ~3.4us. These dummies keep the PE "busy" through
                    # the wait at no cost (it would otherwise stall anyway).
                    for _ in range(6):
                        nc.tensor.matmul(wu_ps[:], wu[:, 0:128], wu[:], start=True, stop=True)
                if bp >= 2:
                    attn_back(bp - 2, prev_front)
                if bp >= 1:
                    prev_front = attn_front(bp - 1, *prev_proj)
                prev_proj = cur
            last_front = attn_front(N_PAIRS - 1, *prev_proj)
            attn_back(N_PAIRS - 2, prev_front)
            attn_back(N_PAIRS - 1, last_front, split_dma=True)

    nc.compile()
    return nc


_CACHE = {}


def get_nc():
    if "nc" not in _CACHE:
        _CACHE["nc"] = _build()
    return _CACHE["nc"]


def make_in_maps(x, Wq, bq, Wk, bk, Wv, bv):
    bf16 = ml_dtypes.bfloat16
    x = np.asarray(x, dtype=np.float32)
    Wq = np.ascontiguousarray(np.asarray(Wq, dtype=np.float32)).astype(bf16)
    Wk = np.ascontiguousarray(np.asarray(Wk, dtype=np.float32)).astype(bf16)
    Wv = np.ascontiguousarray(np.asarray(Wv, dtype=np.float32)).astype(bf16)
    bq = np.ascontiguousarray(np.asarray(bq, dtype=np.float32)).reshape(H, 1)
    bk = np.ascontiguousarray(np.asarray(bk, dtype=np.float32)).reshape(H, 1)
    bv = np.ascontiguousarray(np.asarray(bv, dtype=np.float32)).reshape(H, 1)
    in_maps = []
    for i in range(N_CORES):
        shard = x[i * B_SHARD : (i + 1) * B_SHARD].reshape(B_SHARD * T, C)
        xT = np.ascontiguousarray(shard.T).astype(bf16)  # [C, B_SHARD*T]
        in_maps.append(
            {"xT": xT, "wq": Wq, "wk": Wk, "wv": Wv, "bq": bq, "bk": bk, "bv": bv}
        )
    return in_maps


def kernel(x, Wq, bq, Wk, bk, Wv, bv):
    nc = get_nc()
    in_maps = make_in_maps(x, Wq, bq, Wk, bk, Wv, bv)
    res = bass_utils.run_bass_kernel_spmd(nc, in_maps, core_ids=list(range(N_CORES)))
    # de-swizzle the partition-major device layout: [p, bp, i, n, h] ->
    # [item = bp*2+i, t = n*128+p, h]
    shards = []
    for i in range(N_CORES):
        arr = res.results[i]["out"].reshape(128, N_PAIRS, 2, 2, H)
        shards.append(
            np.transpose(arr, (1, 2, 3, 0, 4)).reshape(B_SHARD, T, H)
        )
    return np.concatenate(shards, axis=0)


# revision 27
# speedup vs baseline: 1.1639x; 1.1639x over previous
"""Causal single-head attention on 8 trn2 cores, batch-data-parallel.

Computes, for each batch item b:
    Q = x[b] @ Wq + bq; K = x[b] @ Wk + bk; V = x[b] @ Wv + bv
    out[b] = softmax(causal_mask(Q K^T / sqrt(H))) @ V

Shapes: x [256, 256, 384], W* [384, 64], b* [64], out [256, 256, 64] fp32.
Sharding: batch axis split across 8 cores (32 items each), weights replicated.
The host feeds x transposed ([C, B*T] layout, bf16) so the kernel needs no
on-device transposition of x (contraction dim C must sit on SBUF partitions).

All matmul operands are bf16 (halves HBM traffic for x, enables fast weight
loads); PSUM accumulation stays fp32. Max elementwise error ~4e-3 vs the fp32
reference, well under the 2e-2 gate.

Batch items are processed in pairs so the projection matmuls stream N=512.
Two-stage software pipeline keeps the PE >90% busy: iteration p emits
  proj(p):       Q/K/V projection matmuls for pair p
  attn_back(p-2): out-matmuls + 1/den normalization + output DMA
  attn_front(p-1): scores, exp+causal-zeroing, V transposes
so the scalar-exp -> gpsimd-zero chain for a pair has a full iteration of
slack before its out-matmuls, and LDWEIGHTS pull-ahead works on the small
matmuls. x^T pair tiles are DMA-prefetched XT_AHEAD pairs ahead; the xt(0)
DMA is issued before everything else (each dma_start costs ~650ns of sync
queue time) and dummy warmup/bridge matmuls keep the PE HAM clock-gate at
2.4 GHz through the initial DMA ramp.

Per pair:
  qv psum [128,512] = rows 0:64 Q^T, rows 64:128 V^T (lhsT=[Wq|Wv], rhs=x^T)
  k  psum [64,512]  = K^T  (lhsT zero-padded to M=128 so FWL stays enabled)
  qv2/k2 sbuf (bf16) = psum + per-partition bias (one DVE op / one ACT op)
Per batch item:
  sT psum [128,3,128] = scores^T blocks [s0,t0],[s0,t1],[s1,t1] (the causally
       dead [s1,t0] block is never computed)
  W = exp(SCALE*sT) straight from PSUM on the scalar engine (bf16), then the
       below-diagonal halves of both diagonal blocks are zeroed in place by
       ONE strided-AP gpsimd affine_select (idle engine, off the DVE)
  V' = [V | 1 | 1] natural layout via PE transposes of V^T; the ones columns
       live in a persistent 6-tile ring (memset once) and make the out matmul
       also produce the softmax denominator (wei @ 1)
  out' = wei^T.T @ V' -> [t, 64 | den | den];  out = out'[:,0:64] * (1/den)
"""

import numpy as np
import ml_dtypes

import concourse.bacc as bacc
import concourse.mybir as mybir
import concourse.tile as tile
from concourse import bass_utils
from concourse.masks import make_identity

N_CORES = 8
B_FULL, T, C, H = 256, 256, 384, 64
B_SHARD = B_FULL // N_CORES  # 32
N_PAIRS = B_SHARD // 2  # 16
F32 = mybir.dt.float32
BF16 = mybir.dt.bfloat16
SCALE = float(H) ** -0.5  # folded into exp: wei = exp(SCALE * scores)

EXP = mybir.ActivationFunctionType.Exp
IS_GE = mybir.AluOpType.is_ge

XT_AHEAD = 4  # pairs of x^T prefetched ahead of the projection stream
WARMUP_MM = 22  # covers PE-preamble-end (~8.4us) to xt(0)-landed (~13us)


def _build():
    nc = bacc.Bacc("TRN2", target_bir_lowering=False, debug=False, num_devices=N_CORES)

    xT_d = nc.dram_tensor("xT", [C, B_SHARD * T], BF16, kind="ExternalInput").ap()
    wq_d = nc.dram_tensor("wq", [C, H], BF16, kind="ExternalInput").ap()
    wk_d = nc.dram_tensor("wk", [C, H], BF16, kind="ExternalInput").ap()
    wv_d = nc.dram_tensor("wv", [C, H], BF16, kind="ExternalInput").ap()
    bq_d = nc.dram_tensor("bq", [H, 1], F32, kind="ExternalInput").ap()
    bk_d = nc.dram_tensor("bk", [H, 1], F32, kind="ExternalInput").ap()
    bv_d = nc.dram_tensor("bv", [H, 1], F32, kind="ExternalInput").ap()
    # partition-major output layout: row p holds every pair's [item, n, h]
    # slice for that partition, so each pair's writeback is one contiguous
    # 1KB segment per partition (4x bigger DMA packets than t-major rows).
    # The host de-swizzles back to [B_SHARD*T, H] for free.
    out_d = nc.dram_tensor(
        "out", [128, N_PAIRS * 2 * 2 * H], F32, kind="ExternalOutput"
    ).ap()

    # x^T per batch pair: [p=c%128, k=c//128, t2=512]
    xT_r = xT_d.rearrange("(k p) (b t) -> b p k t", p=128, b=N_PAIRS)
    # weights: [p=c%128, k=c//128, h]
    wq_r = wq_d.rearrange("(k p) h -> p k h", p=128)
    wk_r = wk_d.rearrange("(k p) h -> p k h", p=128)
    wv_r = wv_d.rearrange("(k p) h -> p k h", p=128)
    # out per pair: element [b, p, i, n, h] is batch item b*2+i, t = n*128+p
    out_r = out_d.rearrange("p (b i n h) -> b p i n h", b=N_PAIRS, i=2, n=2)

    with tile.TileContext(nc) as tc:
        with (
            tc.tile_pool(name="singles", bufs=1) as singles,
            tc.tile_pool(name="sb", bufs=3) as sb,
            tc.tile_pool(name="sbw", bufs=6) as sbw,
            tc.tile_pool(name="sbx", bufs=XT_AHEAD + 1) as sbx,
            tc.tile_pool(name="ps_qv", bufs=2, space="PSUM") as ps_qv,
            tc.tile_pool(name="ps_k", bufs=1, space="PSUM") as ps_k,
            tc.tile_pool(name="ps_s", bufs=2, space="PSUM") as ps_s,
            tc.tile_pool(name="ps_v", bufs=1, space="PSUM") as ps_v,
            tc.tile_pool(name="ps_o", bufs=2, space="PSUM") as ps_o,
        ):
            # ---- DMA issue order: xt(0) first (the critical path), then the
            # weights, then the rest of the prefetch, biases last. Each
            # dma_start costs ~650ns on the sync queue, so order matters.
            xts = {}

            def load_xt(bp):
                t = sbx.tile([128, 3, 512], BF16, tag="xt")
                nc.sync.dma_start(t[:], xT_r[bp])
                xts[bp] = t

            wqv = singles.tile([128, 3, 128], BF16)
            wkk = singles.tile([128, 3, 128], BF16)
            nc.vector.memset(wkk[:], 0.0)
            load_xt(0)
            nc.sync.dma_start(wqv[:, :, 0:64], wq_r)
            nc.sync.dma_start(wqv[:, :, 64:128], wv_r)
            nc.sync.dma_start(wkk[:, :, 0:64], wk_r)
            load_xt(1)
            load_xt(2)
            # per-partition bias vectors: [bq ; bv] and [bk]
            bqv_t = singles.tile([128, 1], F32)
            bk_t = singles.tile([64, 1], F32)
            nc.sync.dma_start(bqv_t[0:64, :], bq_d[:])
            nc.sync.dma_start(bqv_t[64:128, :], bv_d[:])
            nc.sync.dma_start(bk_t[:], bk_d[:])
            for bp in range(3, XT_AHEAD):
                load_xt(bp)

            # ---- one-time setup (no DMA involved) ----
            identf = singles.tile([128, 128], F32)
            make_identity(nc, identf[:])
            ident = singles.tile([128, 128], BF16)
            nc.vector.tensor_copy(ident[:], identf[:])

            # v_sb ring: the ones columns (64:66) are written once here and
            # survive reuse, since the per-item copy only touches cols 0:64.
            v_ring = [
                singles.tile([128, 2, 66], BF16, name=f"v_sb{j}", tag=f"v_sb{j}")
                for j in range(6)
            ]
            for v in v_ring:
                nc.vector.memset(v[:, :, 64:66], 1.0)
            v_ring_i = [0]

            # HAM warmup: the PE clock-gate only opens (1.2 -> 2.4 GHz) after a
            # ~3.4us window of sustained matmul activity. Burn dummy matmuls
            # during the initial DMA wait so the real stream runs warm.
            wu = singles.tile([128, 256], BF16)
            nc.vector.memset(wu[:], 0.0)
            wu_ps = ps_s.tile([128, 256], F32, tag="s_ps")
            for _ in range(WARMUP_MM):
                nc.tensor.matmul(wu_ps[:], wu[:, 0:128], wu[:], start=True, stop=True)
            # (measured: removing this ramp and letting pairs 0-1 warm the
            # clock themselves regresses ~11us -- the dummy ramp is essential.)

            def proj(bp):
                xt = xts.pop(bp)
                qv_ps = ps_qv.tile([128, 512], F32, tag="qv_ps")
                k_ps = ps_k.tile([128, 512], F32, tag="k_ps")
                for c in range(3):
                    nc.tensor.matmul(
                        qv_ps[:], wqv[:, c, :], xt[:, c, :], start=(c == 0), stop=(c == 2)
                    )
                for c in range(3):
                    nc.tensor.matmul(
                        k_ps[:], wkk[:, c, :], xt[:, c, :], start=(c == 0), stop=(c == 2)
                    )
                qv2 = sb.tile([128, 512], BF16, tag="qv2")
                k2 = sb.tile([64, 512], BF16, tag="k2")
                # psum->sbuf bias-add copies, split across DVE and ACT to
                # balance engine load (both are near the critical path)
                nc.vector.tensor_scalar_add(qv2[:], qv_ps[:], bqv_t[:])
                nc.scalar.add(k2[:], k_ps[0:64, :], bk_t[:])
                return qv2, k2

            def attn_front(bp, qv2, k2):
                """Scores, exp+mask, and V transposes for both items of a pair.
                The out-matmuls run one iteration later (attn_back), so the
                exp -> gpsimd-zero chain has a full iteration of slack and the
                PE's LDWEIGHTS pull-ahead works on the out matmuls."""
                items = []
                for bi in range(2):
                    toff = bi * 256
                    qT = qv2[0:64, toff : toff + 256]
                    kT = k2[0:64, toff : toff + 256]

                    # scores^T [s, t] blocks: 0=[s0,t0] 1=[s0,t1] 2=[s1,t1]
                    s_ps = ps_s.tile([128, 3, 128], F32, tag="s_ps")
                    nc.tensor.matmul(
                        s_ps[:, 0:2, :], kT[:, 0:128], qT, start=True, stop=True
                    )
                    nc.tensor.matmul(
                        s_ps[:, 2, :], kT[:, 128:256], qT[:, 128:256], start=True, stop=True
                    )

                    # V natural [s, h] into the ring tile (ones cols persist)
                    v_ps = ps_v.tile([128, 2, 64], BF16, tag="v_ps")
                    for sh in range(2):
                        nc.tensor.transpose(
                            v_ps[:, sh, :],
                            qv2[64:128, toff + sh * 128 : toff + (sh + 1) * 128],
                            ident[64:128, 64:128],
                        )

                    # wei^T = exp(SCALE*scores^T) straight from PSUM (bf16 out),
                    # then one strided-AP affine_select zeroes the below-diag
                    # halves of BOTH diagonal blocks.
                    W = sbw.tile([128, 3, 128], BF16, tag="W")
                    nc.scalar.activation(W[:], s_ps[:], EXP, scale=SCALE)
                    nc.gpsimd.affine_select(
                        out=W[:, 0:3:2, :],
                        in_=W[:, 0:3:2, :],
                        compare_op=IS_GE,
                        fill=0.0,
                        base=0,
                        pattern=[[0, 2], [1, 128]],  # keep where (-s + t) >= 0
                        channel_multiplier=-1,
                    )

                    v_sb = v_ring[v_ring_i[0] % 6]
                    v_ring_i[0] += 1
                    nc.scalar.copy(v_sb[:, :, 0:64], v_ps[:])
                    items.append((W, v_sb))
                return items

            def attn_back(bp, items, split_dma=False):
                osb = sb.tile([128, 2, 2, 64], F32, tag="osb")
                for bi in range(2):
                    W, v_sb = items[bi]
                    # out' = wei^T.T @ [V|1|1] -> [t, 64 | den | den]
                    o_ps = ps_o.tile([128, 2, 66], F32, tag="o_ps")
                    nc.tensor.matmul(o_ps[:, 0, :], W[:, 0, :], v_sb[:, 0, :], start=True, stop=True)
                    nc.tensor.matmul(o_ps[:, 1, :], W[:, 1, :], v_sb[:, 0, :], start=True, stop=False)
                    nc.tensor.matmul(o_ps[:, 1, :], W[:, 2, :], v_sb[:, 1, :], start=False, stop=True)

                    rden = sb.tile([128, 2], F32, tag="rden")
                    nc.vector.reciprocal(rden[:], o_ps[:, :, 64])
                    nc.vector.tensor_scalar_mul(osb[:, bi, 0, :], o_ps[:, 0, 0:64], rden[:, 0:1])
                    nc.vector.tensor_scalar_mul(osb[:, bi, 1, :], o_ps[:, 1, 0:64], rden[:, 1:2])
                    if split_dma:
                        # final pair: overlap item 0's writeback with item 1
                        nc.sync.dma_start(out_r[bp, :, bi], osb[:, bi])
                if not split_dma:
                    nc.sync.dma_start(out_r[bp], osb[:])

            # ---- software-pipelined main loop (2-stage attention):
            # iteration p emits proj(p), out-matmuls(p-2), scores/exp(p-1)
            prev_proj = None
            prev_front = None
            for bp in range(N_PAIRS):
                if bp + XT_AHEAD < N_PAIRS:
                    load_xt(bp + XT_AHEAD)
                cur = proj(bp)
                if bp <= 1:
                    # bridge the early DMA ramp: pairs 0-2 are input-bound, and
                    # a PE idle window here re-throttles the HAM clock to 1.2
                    # GHz for ~3.4us. These dummies keep the PE "busy" through
                    # the wait at no cost (it would otherwise stall anyway).
                    for _ in range(6):
                        nc.tensor.matmul(wu_ps[:], wu[:, 0:128], wu[:], start=True, stop=True)
                if bp >= 2:
                    attn_back(bp - 2, prev_front)
                if bp >= 1:
                    prev_front = attn_front(bp - 1, *prev_proj)
                prev_proj = cur
            last_front = attn_front(N_PAIRS - 1, *prev_proj)
            attn_back(N_PAIRS - 2, prev_front)
            attn_back(N_PAIRS - 1, last_front, split_dma=True)

    nc.compile()
    return nc


_CACHE = {}


def get_nc():
    if "nc" not in _CACHE:
        _CACHE["nc"] = _build()
    return _CACHE["nc"]


def make_in_maps(x, Wq, bq, Wk, bk, Wv, bv):
    bf16 = ml_dtypes.bfloat16
    x = np.asarray(x, dtype=np.float32)
    Wq = np.ascontiguousarray(np.asarray(Wq, dtype=np.float32)).astype(bf16)
    Wk = np.ascontiguousarray(np.asarray(Wk, dtype=np.float32)).astype(bf16)
    Wv = np.ascontiguousarray(np.asarray(Wv, dtype=np.float32)).astype(bf16)
    bq = np.ascontiguousarray(np.asarray(bq, dtype=np.float32)).reshape(H, 1)
    bk = np.ascontiguousarray(np.asarray(bk, dtype=np.float32)).reshape(H, 1)
    bv = np.ascontiguousarray(np.asarray(bv, dtype=np.float32)).reshape(H, 1)
    in_maps = []
    for i in range(N_CORES):
        shard = x[i * B_SHARD : (i + 1) * B_SHARD].reshape(B_SHARD * T, C)
        xT = np.ascontiguousarray(shard.T).astype(bf16)  # [C, B_SHARD*T]
        in_maps.append(
            {"xT": xT, "wq": Wq, "wk": Wk, "wv": Wv, "bq": bq, "bk": bk, "bv": bv}
        )
    return in_maps


def kernel(x, Wq, bq, Wk, bk, Wv, bv):
    nc = get_nc()
    in_maps = make_in_maps(x, Wq, bq, Wk, bk, Wv, bv)
    res = bass_utils.run_bass_kernel_spmd(nc, in_maps, core_ids=list(range(N_CORES)))
    # de-swizzle the partition-major device layout: [p, bp, i, n, h] ->
    # [item = bp*2+i, t = n*128+p, h]
    shards = []
    for i in range(N_CORES):
        arr = res.results[i]["out"].reshape(128, N_PAIRS, 2, 2, H)
        shards.append(
            np.transpose(arr, (1, 2, 3, 0, 4)).reshape(B_SHARD, T, H)
        )
    return np.concatenate(shards, axis=0)
